# revision 28
# baseline (speedup 1.0000x reference)
"""GQA kernel for Trainium2, 8 NeuronCores, tensor-parallel over heads.

Problem: B=1, T=2048, C=4096, 32 q-heads, 16 kv-heads, head_dim=128,
scale = 1/sqrt(32), causal. q head H uses kv head H%16.

Sharding (no collectives needed): core c owns q-heads
{2c, 2c+1, 2c+16, 2c+17} and kv-heads {2c, 2c+1}. Each output column
block depends only on its own head, so the full output is a host-side
concat of per-core column slices.

Per-core kernel (all matmuls bf16, fp32 PSUM accumulation):
  x resident in SBUF as [128, half*32K + kc*1024 + t'] (16 1MB chunk
  tiles; DMA order interleaves the first weight halves with the first
  x chunks so projection matmuls start as soon as chunk 0 lands).
  Startup: q0+k0+v0 projected kc-major with 6 psum accumulators
  (t4 0,1 then 2,3), tracking x-chunk arrival; later units t4-major.
  v strips are PE-transposed into vt ([tk,129] tiles with a ones
  column for the row-sum trick); 4 transposes share one psum bank
  (single-start trick) and drain with one strided DVE copy.
  Attention per head (4 Tq blocks of 512, Tk pairs of 2x128):
    S^T pair = kt^T @ qt -> [128,1024] PSUM, exp (ACT, scale folded),
    causal mask via {0,1} multiply on diagonal tiles (trimmed free
    dims on diagonal pairs), PV: pt slices as stationary, rhs v
    [tk,129]; out normalized by reciprocal(row-sum col).
  attn3 runs blocks 3..0 after q3's t4 3..0 so the kernel tail is the
  smallest block.
"""

import numpy as np
import ml_dtypes

BF16 = ml_dtypes.bfloat16
T = 2048
C = 4096
D = 128
N_HEADS = 32
N_KV_HEADS = 16
SCALE = float(1.0 / np.sqrt(np.float32(N_HEADS)))
KC = C // 128          # 32 contraction chunks
NQH = 4                # local q heads per core
NKV = 2                # local kv heads per core
NT = T // 128          # 16 token tiles
VROW = D + 1           # 129: v with ones column
N_CORES = 8
XCOLS = 2 * KC * 1024

_prog_cache = {}


def _build_program():
    if "nc" in _prog_cache:
        return _prog_cache["nc"]
    import concourse.bass as bass
    import concourse.tile as tile
    from concourse import bacc, mybir

    dt = mybir.dt
    f32 = dt.float32
    bf16 = dt.bfloat16
    EXP = mybir.ActivationFunctionType.Exp

    nc = bacc.Bacc("TRN2", target_bir_lowering=False, debug=False,
                   num_devices=N_CORES)

    xh_d = nc.dram_tensor("xh", [128, XCOLS], bf16, kind="ExternalInput").ap()
    wq_d = nc.dram_tensor("wq", [NQH, 128, C], bf16, kind="ExternalInput").ap()
    wk_d = nc.dram_tensor("wk", [NKV, 128, C], bf16, kind="ExternalInput").ap()
    wv_d = nc.dram_tensor("wv", [NKV, 128, C], bf16, kind="ExternalInput").ap()
    # masks: 4x [128,512] causal tiles + [128,128] identity for PE transpose
    mask_d = nc.dram_tensor("masks", [128, 4 * 512 + 128], bf16,
                            kind="ExternalInput").ap()
    out_d = nc.dram_tensor("out", [T, NQH * D], f32, kind="ExternalOutput").ap()

    with tile.TileContext(nc) as tc:
        with (
            tc.tile_pool(name="persist", bufs=1) as persist,
            tc.tile_pool(name="wpool", bufs=3) as wpool,
            tc.tile_pool(name="vtsp", bufs=1) as vtsp,
            tc.tile_pool(name="ptpool", bufs=3) as ptpool,
            tc.tile_pool(name="opool", bufs=8) as opool,
            tc.tile_pool(name="recpool", bufs=8) as recpool,
            tc.tile_pool(name="psum", bufs=4, space=bass.MemorySpace.PSUM) as psum,
            tc.tile_pool(name="psum2", bufs=2, space=bass.MemorySpace.PSUM) as psum2,
        ):
            mask_sb = persist.tile([128, 4 * 512 + 128], bf16, name="mask_sb",
                                   tag="mask_sb")
            ident = mask_sb[:, 4 * 512: 4 * 512 + 128]

            qt = persist.tile([128, NQH * T], bf16, name="qt", tag="qt")
            kt = persist.tile([128, NKV * T], bf16, name="kt", tag="kt")
            vt = persist.tile([128, NKV * NT * VROW], bf16, name="vt", tag="vt")

            # ones columns of v (row-sum trick)
            for i in range(NKV * NT):
                nc.vector.memset(vt[:, i * VROW + D: (i + 1) * VROW], 1.0)

            # x chunk layout: (half, kc_lo, n_kc); first two chunks smaller
            # so the first matmuls start as early as possible.
            xchunks = ([(0, 0, 2), (0, 2, 2)]
                       + [(0, k, 4) for k in range(4, KC, 4)]
                       + [(1, k, 4) for k in range(0, KC, 4)])
            xmap = {}
            xtiles = {}
            wts = {}

            def dma_x(ci):
                half, klo, nk = xchunks[ci]
                xc = persist.tile([128, nk * 1024], bf16, name=f"xc{ci}",
                                  tag=f"xc{ci}")
                base = half * (KC * 1024) + klo * 1024
                nc.sync.dma_start(out=xc[:],
                                  in_=xh_d[:, base:base + nk * 1024])
                xtiles[ci] = xc
                for k in range(klo, klo + nk):
                    xmap[(half, k)] = (ci, (k - klo) * 1024)

            def xs(t4, kc):
                ci, off = xmap[(t4 // 2, kc)]
                off += (t4 % 2) * 512
                return xtiles[ci][:, off:off + 512]

            def dma_w(src, idx, key, cols=None):
                if key not in wts:
                    w = wpool.tile([128, C], bf16, name=f"w_{key}", tag="w")
                    wts[key] = w
                w = wts[key]
                if cols is None:
                    nc.sync.dma_start(out=w[:], in_=src[idx])
                else:
                    lo, hi = cols
                    nc.sync.dma_start(out=w[:, lo:hi], in_=src[idx][:, lo:hi])

            def emit_tr_group(vts, kv, t4):
                """PE-transpose 4 [128,128] v tiles into one psum bank
                (single-start trick), one strided DVE copy into vt."""
                trp = psum.tile([128, 512], bf16, name=f"tr_{kv}_{t4}",
                                tag="ps")
                for rr in range(4):
                    j = t4 * 4 + rr
                    nc.tensor.matmul(
                        trp[:, rr * 128:(rr + 1) * 128],
                        lhsT=vts[:, j * 128:(j + 1) * 128],
                        rhs=ident, is_transpose=True,
                        start=(rr == 0), stop=(rr == 3),
                        skip_group_check=True)
                for rr in range(4):
                    j = t4 * 4 + rr
                    nc.vector.tensor_copy(
                        out=vt[:, (kv * NT + j) * VROW:
                               (kv * NT + j) * VROW + D],
                        in_=trp[:, rr * 128:(rr + 1) * 128])

            def startup():
                """q0+k0+v0 kc-major with 6 accumulators, per x half."""
                wq0, wk0, wv0 = wts["q0"], wts["k0"], wts["v0"]
                vts = vtsp.tile([128, T], bf16, name="vts_0", tag="vts")
                for half in (0, 1):
                    aq = psum2.tile([128, 1024], f32, name=f"aq{half}",
                                    tag="sp2")
                    accq = [aq[:, 0:512], aq[:, 512:1024]]
                    acck = [psum.tile([128, 512], f32, name=f"ak{half}{i}",
                                      tag="ps")[:] for i in (0, 1)]
                    accv = [psum.tile([128, 512], f32, name=f"av{half}{i}",
                                      tag="ps")[:] for i in (0, 1)]
                    for kc in range(KC):
                        for acc, w in ((accq, wq0), (acck, wk0), (accv, wv0)):
                            for i in (0, 1):
                                nc.tensor.matmul(
                                    acc[i],
                                    lhsT=w[:, kc * 128:(kc + 1) * 128],
                                    rhs=xs(2 * half + i, kc),
                                    start=(kc == 0), stop=(kc == KC - 1))
                    for i in (0, 1):
                        t4 = 2 * half + i
                        nc.vector.tensor_copy(
                            out=qt[:, t4 * 512:(t4 + 1) * 512], in_=accq[i])
                        nc.vector.tensor_copy(
                            out=kt[:, t4 * 512:(t4 + 1) * 512], in_=acck[i])
                        nc.vector.tensor_copy(
                            out=vts[:, t4 * 512:(t4 + 1) * 512], in_=accv[i])
                    emit_tr_group(vts, 0, 2 * half)
                    emit_tr_group(vts, 0, 2 * half + 1)

            def proj(wkey, dest, dbase, vts_kv=None, t4_order=(0, 1, 2, 3)):
                """t4-major projection of one [128, T] strip."""
                w = wts[wkey]
                vts = None
                if vts_kv is not None:
                    vts = vtsp.tile([128, T], bf16, name=f"vts_{wkey}",
                                    tag="vts")
                if True:
                    for t4 in t4_order:
                        ps = psum.tile([128, 512], f32,
                                       name=f"ps_{wkey}_{t4}", tag="ps")
                        for kc in range(KC):
                            nc.tensor.matmul(
                                ps[:], lhsT=w[:, kc * 128:(kc + 1) * 128],
                                rhs=xs(t4, kc),
                                start=(kc == 0), stop=(kc == KC - 1))
                        if vts is None:
                            nc.vector.tensor_copy(
                                out=dest[:, dbase + t4 * 512:
                                         dbase + (t4 + 1) * 512],
                                in_=ps[:])
                        else:
                            nc.vector.tensor_copy(
                                out=vts[:, t4 * 512:(t4 + 1) * 512], in_=ps[:])
                    if vts is not None:
                        for t4 in range(4):
                            emit_tr_group(vts, vts_kv, t4)

            def attn(h, hooks=None, block_order=(0, 1, 2, 3)):
                """Generator, yields per pair-step. Software-pipelined:
                step i emits S/exp/mask of pair i and the PV matmuls of
                pair i-1 (lag crosses block boundaries), so PV weight
                loads never wait on a just-issued exp. pvs[0],pvs[1] share
                a claimed psum bank; pvs[2],pvs[3] own banks."""
                kv = h % 2
                zeros259 = mask_sb[:, 3 * 512:3 * 512 + 259]
                state = {}  # per-block psum tiles, created at first PV

                def drain(b, s, pv, cb):
                    rec = recpool.tile([128, 1], f32,
                                       name=f"rec_{h}_{b}_{s}", tag="rec")
                    nc.vector.reciprocal(rec[:], pv[:, cb + D:cb + D + 1])
                    ot = opool.tile([128, 128], f32,
                                    name=f"ot_{h}_{b}_{s}", tag="ot")
                    nc.vector.tensor_scalar_mul(ot[:], pv[:, cb:cb + D], rec[:])
                    nc.sync.dma_start(
                        out=out_d[b * 512 + s * 128:b * 512 + (s + 1) * 128,
                                  h * D:(h + 1) * D],
                        in_=ot[:])

                def emit_s(b, p):
                    spp = psum2.tile([128, 1024], f32,
                                     name=f"sp_{h}_{b}_{p}", tag="sp2")
                    pt = ptpool.tile([128, 1024], bf16,
                                     name=f"pt_{h}_{b}_{p}", tag="pt")
                    qsl = qt[:, h * T + b * 512:h * T + (b + 1) * 512]
                    for half in range(2):
                        j = 2 * p + half
                        nc.tensor.matmul(
                            spp[:, half * 512:(half + 1) * 512],
                            lhsT=kt[:, kv * T + j * 128:kv * T + (j + 1) * 128],
                            rhs=qsl, start=True, stop=True)
                    nc.scalar.activation(pt[:], spp[:], EXP, scale=SCALE)
                    if p >= 2 * b:
                        roff = (p - 2 * b) * 1024
                        nc.vector.tensor_mul(pt[:], pt[:],
                                             mask_sb[:, roff:roff + 1024])
                    return pt

                def emit_pv(b, p, pt, bi):
                    if b not in state:
                        pvA = psum.tile([128, 512], f32,
                                        name=f"pvA_{h}_{b}", tag="ps")
                        nc.tensor.matmul(pvA[:, 0:259], lhsT=ident,
                                         rhs=zeros259, start=True, stop=False,
                                         skip_group_check=True)
                        pvs23 = [psum.tile([128, 512], f32,
                                           name=f"pv_{h}_{b}_{s}", tag="ps")
                                 for s in (2, 3)]
                        state[b] = (pvA, pvs23)
                    pvA, pvs23 = state[b]
                    for half in range(2):
                        j = 2 * p + half
                        r = j - 4 * b
                        vsl = vt[:, (kv * NT + j) * VROW:
                                 (kv * NT + j + 1) * VROW]
                        for s in range(max(0, r), 4):
                            if s < 2:
                                nc.tensor.matmul(
                                    pvA[:, 130 * s:130 * s + VROW],
                                    lhsT=pt[:, half * 512 + s * 128:
                                            half * 512 + (s + 1) * 128],
                                    rhs=vsl, start=False, stop=False,
                                    skip_group_check=True)
                            else:
                                nc.tensor.matmul(
                                    pvs23[s - 2][:, 0:VROW],
                                    lhsT=pt[:, half * 512 + s * 128:
                                            half * 512 + (s + 1) * 128],
                                    rhs=vsl, start=(j == 0),
                                    stop=(j == 4 * b + s))
                    if p == 2 * b:
                        drain(b, 0, pvA, 0)
                        drain(b, 1, pvA, 130)
                    elif p == 2 * b + 1:
                        drain(b, 2, pvs23[0], 0)
                        drain(b, 3, pvs23[1], 0)
                        del state[b]
                        if hooks and bi in hooks:
                            hooks[bi]()

                steps = [(bi, b, p) for bi, b in enumerate(block_order)
                         for p in range(2 * b + 2)]
                prev = None
                for bi, b, p in steps:
                    pt = emit_s(b, p)
                    if prev is not None:
                        emit_pv(*prev)
                    prev = (b, p, pt, bi)
                    yield
                emit_pv(*prev)
                yield

            def proj(wkey, dest, dbase, vts_kv=None, t4_order=(0, 1, 2, 3)):
                """t4-major projection of one [128, T] strip."""
                w = wts[wkey]
                vts = None
                if vts_kv is not None:
                    vts = vtsp.tile([128, T], bf16, name=f"vts_{wkey}",
                                    tag="vts")
                if True:
                    for t4 in t4_order:
                        ps = psum.tile([128, 512], f32,
                                       name=f"ps_{wkey}_{t4}", tag="ps")
                        for kc in range(KC):
                            nc.tensor.matmul(
                                ps[:], lhsT=w[:, kc * 128:(kc + 1) * 128],
                                rhs=xs(t4, kc),
                                start=(kc == 0), stop=(kc == KC - 1))
                        if vts is None:
                            nc.vector.tensor_copy(
                                out=dest[:, dbase + t4 * 512:
                                         dbase + (t4 + 1) * 512],
                                in_=ps[:])
                        else:
                            nc.vector.tensor_copy(
                                out=vts[:, t4 * 512:(t4 + 1) * 512], in_=ps[:])
                    if vts is not None:
                        for t4 in range(4):
                            emit_tr_group(vts, vts_kv, t4)

            def attn(h, hooks=None, block_order=(0, 1, 2, 3)):
                """Generator: yields after each S/PV pair. pvs[0],pvs[1]
                share one psum bank (claimed by a zeroing matmul so all
                PV accumulations run start=False); pvs[2],pvs[3] own banks."""
                kv = h % 2
                zeros259 = mask_sb[:, 3 * 512:3 * 512 + 259]

                def drain(b, s, pv, cb):
                    rec = recpool.tile([128, 1], f32,
                                       name=f"rec_{h}_{b}_{s}", tag="rec")
                    nc.vector.reciprocal(rec[:], pv[:, cb + D:cb + D + 1])
                    ot = opool.tile([128, 128], f32,
                                    name=f"ot_{h}_{b}_{s}", tag="ot")
                    nc.vector.tensor_scalar_mul(ot[:], pv[:, cb:cb + D], rec[:])
                    nc.sync.dma_start(
                        out=out_d[b * 512 + s * 128:b * 512 + (s + 1) * 128,
                                  h * D:(h + 1) * D],
                        in_=ot[:])

                if True:
                    for bi, b in enumerate(block_order):
                        pvA = psum.tile([128, 512], f32,
                                        name=f"pvA_{h}_{b}", tag="ps")
                        nc.tensor.matmul(pvA[:, 0:259], lhsT=ident,
                                         rhs=zeros259, start=True, stop=False,
                                         skip_group_check=True)
                        pvs23 = [psum.tile([128, 512], f32,
                                           name=f"pv_{h}_{b}_{s}", tag="ps")
                                 for s in (2, 3)]
                        for p in range(2 * b + 2):  # pairs of Tk tiles
                            spp = psum2.tile([128, 1024], f32,
                                             name=f"sp_{h}_{b}_{p}", tag="sp2")
                            pt = ptpool.tile([128, 1024], bf16,
                                             name=f"pt_{h}_{b}_{p}", tag="pt")
                            qsl = qt[:, h * T + b * 512:h * T + (b + 1) * 512]
                            for half in range(2):
                                j = 2 * p + half
                                nc.tensor.matmul(
                                    spp[:, half * 512:(half + 1) * 512],
                                    lhsT=kt[:, kv * T + j * 128:
                                            kv * T + (j + 1) * 128],
                                    rhs=qsl,
                                    start=True, stop=True,
                                )
                            nc.scalar.activation(pt[:], spp[:], EXP,
                                                 scale=SCALE)
                            if p >= 2 * b:  # diagonal pairs
                                roff = (p - 2 * b) * 1024
                                nc.vector.tensor_mul(
                                    pt[:], pt[:],
                                    mask_sb[:, roff:roff + 1024])
                            for half in range(2):
                                j = 2 * p + half
                                r = j - 4 * b
                                vsl = vt[:, (kv * NT + j) * VROW:
                                         (kv * NT + j + 1) * VROW]
                                for s in range(max(0, r), 4):
                                    if s < 2:
                                        nc.tensor.matmul(
                                            pvA[:, 130 * s:130 * s + VROW],
                                            lhsT=pt[:, half * 512 + s * 128:
                                                    half * 512 + (s + 1) * 128],
                                            rhs=vsl,
                                            start=False, stop=False,
                                            skip_group_check=True,
                                        )
                                    else:
                                        nc.tensor.matmul(
                                            pvs23[s - 2][:, 0:VROW],
                                            lhsT=pt[:, half * 512 + s * 128:
                                                    half * 512 + (s + 1) * 128],
                                            rhs=vsl,
                                            start=(j == 0),
                                            stop=(j == 4 * b + s),
                                        )
                            if p == 2 * b:
                                drain(b, 0, pvA, 0)
                                drain(b, 1, pvA, 130)
                            elif p == 2 * b + 1:
                                drain(b, 2, pvs23[0], 0)
                                drain(b, 3, pvs23[1], 0)
                            yield
                        if hooks and bi in hooks:
                            hooks[bi]()

            def proj(wkey, dest, dbase, vts_kv=None, t4_order=(0, 1, 2, 3)):
                """Generator: t4-major projection, yields every 8 kc."""
                w = wts[wkey]
                vts = None
                if vts_kv is not None:
                    vts = vtsp.tile([128, T], bf16, name=f"vts_{wkey}",
                                    tag="vts")
                if True:
                    for t4 in t4_order:
                        ps = psum.tile([128, 512], f32,
                                       name=f"ps_{wkey}_{t4}", tag="ps")
                        for kc in range(KC):
                            nc.tensor.matmul(
                                ps[:], lhsT=w[:, kc * 128:(kc + 1) * 128],
                                rhs=xs(t4, kc),
                                start=(kc == 0), stop=(kc == KC - 1))
                            if kc % 8 == 7 and kc != KC - 1:
                                yield
                        if vts is None:
                            nc.vector.tensor_copy(
                                out=dest[:, dbase + t4 * 512:
                                         dbase + (t4 + 1) * 512],
                                in_=ps[:])
                        else:
                            nc.vector.tensor_copy(
                                out=vts[:, t4 * 512:(t4 + 1) * 512], in_=ps[:])
                        yield
                    if vts is not None:
                        for t4 in range(4):
                            emit_tr_group(vts, vts_kv, t4)
                            yield

            def weave(*streams, until=None):
                """Round-robin weighted interleave. If until is given (index
                into streams), stop as soon as that stream exhausts, leaving
                the other generators unconsumed for later weaves."""
                live = [[g, wt] for g, wt in streams]
                gens = [g for g, _ in streams]
                while live:
                    done = []
                    for ent in live:
                        g, wt = ent
                        for _ in range(wt):
                            try:
                                next(g)
                            except StopIteration:
                                done.append(ent)
                                break
                    for ent in done:
                        live.remove(ent)
                        if until is not None and ent[0] is gens[until]:
                            return

            def chain(*gens):
                for g in gens:
                    yield from g

            # ---- DMA schedule (ring is FIFO in issue order) ----
            # Weight eighths (4 kc each) for the three startup units
            # interleaved with the x chunks that consume them; startup
            # compute is PE-bound from the first chunk.
            E = C // 8
            # h0 x chunk index covering kc group g: chunks 0,1 are kc0-1,2-3
            h0_chunk_for_g = {0: (0, 1), 1: (2,), 2: (3,), 3: (4,),
                              4: (5,), 5: (6,), 6: (7,), 7: (8,)}
            xi_done = set()
            for g in range(8):
                for key, src in (("q0", wq_d), ("k0", wk_d), ("v0", wv_d)):
                    dma_w(src, 0, key, cols=(g * E, (g + 1) * E))
                for ci in h0_chunk_for_g[g]:
                    if ci not in xi_done:
                        dma_x(ci)
                        xi_done.add(ci)
                if g == 3:
                    nc.sync.dma_start(out=mask_sb[:], in_=mask_d[:])
            for ci in range(9, 17):
                dma_x(ci)

            # ---- compute schedule ----
            # attn heads weave only with units they do not depend on;
            # attn3 (blocks 3..0) weaves with q3's t4 1,0 after t4 3,2
            # completed during the attn1 weave.
            startup()
            dma_w(wq_d, 2, "q2")
            units_a = chain(proj("q2", qt, 2 * T), proj("k1", kt, T))
            weave((attn(0, hooks={1: lambda: dma_w(wk_d, 1, "k1"),
                                  2: lambda: dma_w(wv_d, 1, "v1")}), 1),
                  (units_a, 1), until=0)
            dma_w(wq_d, 1, "q1")
            weave((attn(2, hooks={2: lambda: dma_w(wq_d, 3, "q3")}), 1),
                  (chain(units_a,
                         proj("v1", None, 0, vts_kv=1),
                         proj("q1", qt, T)), 2))
            weave((attn(1), 1),
                  (proj("q3", qt, 3 * T, t4_order=(3, 2)), 1))
            weave((attn(3, block_order=(3, 2, 1, 0)), 1),
                  (proj("q3", qt, 3 * T, t4_order=(1, 0)), 1))

    nc.compile()
    _prog_cache["nc"] = nc
    return nc


def _host_prep(x, Wq, bq, Wk, bk, Wv, bv):
    """Shard + repack inputs for the 8 cores. Returns in_maps list."""
    assert x.shape == (1, T, C)
    assert np.abs(bq).max() == 0 and np.abs(bk).max() == 0, \
        "nonzero q/k biases not supported"

    x0 = np.ascontiguousarray(x[0]).astype(BF16)
    # xh packed: [128, half*32K + kc*1024 + t'] = x[half*1024+t', kc*128+p]
    xh = np.ascontiguousarray(
        x0.reshape(2, 1024, KC, 128).transpose(3, 0, 2, 1).reshape(128, XCOLS))

    # causal masks for the 4 diagonal-tile offsets: mask_r[tk,tq] = tq >= tk+128r
    tq = np.arange(512)[None, :]
    tk = np.arange(128)[:, None]
    masks = np.concatenate(
        [(tq >= (tk + 128 * r)).astype(BF16) for r in range(4)]
        + [np.eye(128, dtype=BF16)], axis=1)
    masks = np.ascontiguousarray(masks)

    def pack_w(Wrows):
        # Wrows: [128 (out c), C (in)] for one head ->
        # packed[p, 128*kc + c] = Wrows[c, 128*kc + p]
        return np.ascontiguousarray(
            Wrows.astype(BF16).reshape(128, KC, 128).transpose(2, 1, 0)
            .reshape(128, C))

    in_maps = []
    for c in range(N_CORES):
        qheads = [2 * c, 2 * c + 1, 2 * c + 16, 2 * c + 17]
        kvheads = [2 * c, 2 * c + 1]
        wq = np.stack([pack_w(Wq[128 * H:128 * (H + 1)]) for H in qheads])
        wk = np.stack([pack_w(Wk[128 * K:128 * (K + 1)]) for K in kvheads])
        wv = np.stack([pack_w(Wv[128 * K:128 * (K + 1)]) for K in kvheads])
        in_maps.append({
            "xh": xh, "wq": wq, "wk": wk, "wv": wv, "masks": masks,
        })
    return in_maps


def _assemble(results, bv):
    out = np.empty((T, C), dtype=np.float32)
    for c in range(N_CORES):
        r = results[c]["out"]
        qheads = [2 * c, 2 * c + 1, 2 * c + 16, 2 * c + 17]
        for i, H in enumerate(qheads):
            blk = r[:, 128 * i:128 * (i + 1)]
            if bv is not None:
                blk = blk + bv[128 * (H % N_KV_HEADS):
                               128 * (H % N_KV_HEADS) + 128]
            out[:, 128 * H:128 * (H + 1)] = blk
    return out.reshape(1, T, C)


def _install_trace_hooks():
    """The agent image's antenv lacks axon_hooks; recreate it so
    run_bass_kernel_spmd's trace=True path can capture NTFF profiles."""
    import sys
    import types
    import antenv
    if "antenv.axon_hooks" not in sys.modules:
        mod = types.ModuleType("antenv.axon_hooks")
        mod._hook = None

        def set_axon_ntff_profile_hook(h):
            mod._hook = h

        def get_axon_ntff_profile_hook():
            return mod._hook

        mod.set_axon_ntff_profile_hook = set_axon_ntff_profile_hook
        mod.get_axon_ntff_profile_hook = get_axon_ntff_profile_hook
        sys.modules["antenv.axon_hooks"] = mod
        antenv.axon_hooks = mod
    from antenv.axon_hooks import (get_axon_ntff_profile_hook,
                                   set_axon_ntff_profile_hook)
    if get_axon_ntff_profile_hook() is None:
        if "/root/.axon_site" not in sys.path:
            sys.path.insert(0, "/root/.axon_site")
        from trn_agent_boot.trn_boot import _ntff_profile_via_ctypes
        set_axon_ntff_profile_hook(
            _ntff_profile_via_ctypes("/opt/axon/libaxon_pjrt.so"))
    import concourse.bass_utils as bu
    bu.upload_artifacts = lambda tmpdir: tmpdir


def _run(inputs, trace=False, trace_kwargs=None):
    if trace:
        _install_trace_hooks()
    from concourse.bass_utils import run_bass_kernel_spmd
    nc = _build_program()
    in_maps = _host_prep(**inputs)
    res = run_bass_kernel_spmd(
        nc, in_maps, list(range(N_CORES)), trace=trace,
        **(trace_kwargs or {}))
    bv = inputs["bv"].astype(np.float32)
    bv = bv if np.abs(bv).max() > 0 else None
    out = _assemble(res.results, bv)
    return out, res


def kernel(x, Wq, bq, Wk, bk, Wv, bv):
    out, _ = _run(dict(x=np.asarray(x), Wq=np.asarray(Wq), bq=np.asarray(bq),
                       Wk=np.asarray(Wk), bk=np.asarray(bk),
                       Wv=np.asarray(Wv), bv=np.asarray(bv)))
    return out


# revision 29
# speedup vs baseline: 1.0159x; 1.0159x over previous
"""GQA kernel for Trainium2, 8 NeuronCores, tensor-parallel over heads.

Problem: B=1, T=2048, C=4096, 32 q-heads, 16 kv-heads, head_dim=128,
scale = 1/sqrt(32), causal. q head H uses kv head H%16.

Sharding (no collectives needed): core c owns q-heads
{2c, 2c+1, 2c+16, 2c+17} and kv-heads {2c, 2c+1}. Each output column
block depends only on its own head, so the full output is a host-side
concat of per-core column slices.

Per-core kernel (all matmuls bf16, fp32 PSUM accumulation):
  x resident in SBUF as [128, half*32K + kc*1024 + t'] (16 1MB chunk
  tiles; DMA order interleaves the first weight halves with the first
  x chunks so projection matmuls start as soon as chunk 0 lands).
  Startup: q0+k0+v0 projected kc-major with 6 psum accumulators
  (t4 0,1 then 2,3), tracking x-chunk arrival; later units t4-major.
  v strips are PE-transposed into vt ([tk,129] tiles with a ones
  column for the row-sum trick); 4 transposes share one psum bank
  (single-start trick) and drain with one strided DVE copy.
  Attention per head (4 Tq blocks of 512, Tk pairs of 2x128):
    S^T pair = kt^T @ qt -> [128,1024] PSUM, exp (ACT, scale folded),
    causal mask via {0,1} multiply on diagonal tiles (trimmed free
    dims on diagonal pairs), PV: pt slices as stationary, rhs v
    [tk,129]; out normalized by reciprocal(row-sum col).
  attn3 runs blocks 3..0 after q3's t4 3..0 so the kernel tail is the
  smallest block.
"""

import numpy as np
import ml_dtypes

BF16 = ml_dtypes.bfloat16
T = 2048
C = 4096
D = 128
N_HEADS = 32
N_KV_HEADS = 16
SCALE = float(1.0 / np.sqrt(np.float32(N_HEADS)))
KC = C // 128          # 32 contraction chunks
NQH = 4                # local q heads per core
NKV = 2                # local kv heads per core
NT = T // 128          # 16 token tiles
VROW = D + 1           # 129: v with ones column
N_CORES = 8
XCOLS = 2 * KC * 1024

_prog_cache = {}


def _build_program():
    if "nc" in _prog_cache:
        return _prog_cache["nc"]
    import concourse.bass as bass
    import concourse.tile as tile
    from concourse import bacc, mybir

    dt = mybir.dt
    f32 = dt.float32
    bf16 = dt.bfloat16
    EXP = mybir.ActivationFunctionType.Exp

    nc = bacc.Bacc("TRN2", target_bir_lowering=False, debug=False,
                   num_devices=N_CORES)

    xh_d = nc.dram_tensor("xh", [128, XCOLS], bf16, kind="ExternalInput").ap()
    wq_d = nc.dram_tensor("wq", [NQH, 128, C], bf16, kind="ExternalInput").ap()
    wk_d = nc.dram_tensor("wk", [NKV, 128, C], bf16, kind="ExternalInput").ap()
    wv_d = nc.dram_tensor("wv", [NKV, 128, C], bf16, kind="ExternalInput").ap()
    # masks: 4x [128,512] causal tiles + [128,128] identity for PE transpose
    mask_d = nc.dram_tensor("masks", [128, 4 * 512 + 128], bf16,
                            kind="ExternalInput").ap()
    out_d = nc.dram_tensor("out", [T, NQH * D], f32, kind="ExternalOutput").ap()

    with tile.TileContext(nc) as tc:
        with (
            tc.tile_pool(name="persist", bufs=1) as persist,
            tc.tile_pool(name="wpool", bufs=3) as wpool,
            tc.tile_pool(name="vtsp", bufs=1) as vtsp,
            tc.tile_pool(name="ptpool", bufs=3) as ptpool,
            tc.tile_pool(name="opool", bufs=8) as opool,
            tc.tile_pool(name="recpool", bufs=8) as recpool,
            tc.tile_pool(name="psum", bufs=4, space=bass.MemorySpace.PSUM) as psum,
            tc.tile_pool(name="psum2", bufs=2, space=bass.MemorySpace.PSUM) as psum2,
        ):
            mask_sb = persist.tile([128, 4 * 512 + 128], bf16, name="mask_sb",
                                   tag="mask_sb")
            ident = mask_sb[:, 4 * 512: 4 * 512 + 128]

            qt = persist.tile([128, NQH * T], bf16, name="qt", tag="qt")
            kt = persist.tile([128, NKV * T], bf16, name="kt", tag="kt")
            vt = persist.tile([128, NKV * NT * VROW], bf16, name="vt", tag="vt")

            # ones columns of v (row-sum trick)
            for i in range(NKV * NT):
                nc.vector.memset(vt[:, i * VROW + D: (i + 1) * VROW], 1.0)

            # x chunk layout: (half, kc_lo, n_kc); first two chunks smaller
            # so the first matmuls start as early as possible.
            xchunks = ([(0, 0, 2), (0, 2, 2)]
                       + [(0, k, 4) for k in range(4, KC, 4)]
                       + [(1, k, 4) for k in range(0, KC, 4)])
            xmap = {}
            xtiles = {}
            wts = {}

            def dma_x(ci):
                half, klo, nk = xchunks[ci]
                xc = persist.tile([128, nk * 1024], bf16, name=f"xc{ci}",
                                  tag=f"xc{ci}")
                base = half * (KC * 1024) + klo * 1024
                nc.sync.dma_start(out=xc[:],
                                  in_=xh_d[:, base:base + nk * 1024])
                xtiles[ci] = xc
                for k in range(klo, klo + nk):
                    xmap[(half, k)] = (ci, (k - klo) * 1024)

            def xs(t4, kc):
                ci, off = xmap[(t4 // 2, kc)]
                off += (t4 % 2) * 512
                return xtiles[ci][:, off:off + 512]

            def dma_w(src, idx, key, cols=None):
                if key not in wts:
                    w = wpool.tile([128, C], bf16, name=f"w_{key}", tag="w")
                    wts[key] = w
                w = wts[key]
                if cols is None:
                    nc.sync.dma_start(out=w[:], in_=src[idx])
                else:
                    lo, hi = cols
                    nc.sync.dma_start(out=w[:, lo:hi], in_=src[idx][:, lo:hi])

            def emit_tr_group(vts, kv, t4):
                """PE-transpose 4 [128,128] v tiles into one psum bank
                (single-start trick), one strided DVE copy into vt."""
                trp = psum.tile([128, 512], bf16, name=f"tr_{kv}_{t4}",
                                tag="ps")
                for rr in range(4):
                    j = t4 * 4 + rr
                    nc.tensor.matmul(
                        trp[:, rr * 128:(rr + 1) * 128],
                        lhsT=vts[:, j * 128:(j + 1) * 128],
                        rhs=ident, is_transpose=True,
                        start=(rr == 0), stop=(rr == 3),
                        skip_group_check=True)
                for rr in range(4):
                    j = t4 * 4 + rr
                    nc.vector.tensor_copy(
                        out=vt[:, (kv * NT + j) * VROW:
                               (kv * NT + j) * VROW + D],
                        in_=trp[:, rr * 128:(rr + 1) * 128])

            def startup():
                """q0+k0+v0 kc-major with 6 accumulators, per x half."""
                wq0, wk0, wv0 = wts["q0"], wts["k0"], wts["v0"]
                vts = vtsp.tile([128, T], bf16, name="vts_0", tag="vts")
                for half in (0, 1):
                    aq = psum2.tile([128, 1024], f32, name=f"aq{half}",
                                    tag="sp2")
                    accq = [aq[:, 0:512], aq[:, 512:1024]]
                    acck = [psum.tile([128, 512], f32, name=f"ak{half}{i}",
                                      tag="ps")[:] for i in (0, 1)]
                    accv = [psum.tile([128, 512], f32, name=f"av{half}{i}",
                                      tag="ps")[:] for i in (0, 1)]
                    for kc in range(KC):
                        for acc, w in ((accq, wq0), (acck, wk0), (accv, wv0)):
                            for i in (0, 1):
                                nc.tensor.matmul(
                                    acc[i],
                                    lhsT=w[:, kc * 128:(kc + 1) * 128],
                                    rhs=xs(2 * half + i, kc),
                                    start=(kc == 0), stop=(kc == KC - 1))
                    for i in (0, 1):
                        t4 = 2 * half + i
                        nc.vector.tensor_copy(
                            out=qt[:, t4 * 512:(t4 + 1) * 512], in_=accq[i])
                        nc.vector.tensor_copy(
                            out=kt[:, t4 * 512:(t4 + 1) * 512], in_=acck[i])
                        nc.vector.tensor_copy(
                            out=vts[:, t4 * 512:(t4 + 1) * 512], in_=accv[i])
                    emit_tr_group(vts, 0, 2 * half)
                    emit_tr_group(vts, 0, 2 * half + 1)

            def proj(wkey, dest, dbase, vts_kv=None, t4_order=(0, 1, 2, 3)):
                """t4-major projection of one [128, T] strip."""
                w = wts[wkey]
                vts = None
                if vts_kv is not None:
                    vts = vtsp.tile([128, T], bf16, name=f"vts_{wkey}",
                                    tag="vts")
                if True:
                    for t4 in t4_order:
                        ps = psum.tile([128, 512], f32,
                                       name=f"ps_{wkey}_{t4}", tag="ps")
                        for kc in range(KC):
                            nc.tensor.matmul(
                                ps[:], lhsT=w[:, kc * 128:(kc + 1) * 128],
                                rhs=xs(t4, kc),
                                start=(kc == 0), stop=(kc == KC - 1))
                        if vts is None:
                            nc.vector.tensor_copy(
                                out=dest[:, dbase + t4 * 512:
                                         dbase + (t4 + 1) * 512],
                                in_=ps[:])
                        else:
                            nc.vector.tensor_copy(
                                out=vts[:, t4 * 512:(t4 + 1) * 512], in_=ps[:])
                    if vts is not None:
                        for t4 in range(4):
                            emit_tr_group(vts, vts_kv, t4)

            def attn(h, hooks=None, block_order=(0, 1, 2, 3)):
                """Generator, yields per pair-step. Software-pipelined:
                step i emits S/exp/mask of pair i and the PV matmuls of
                pair i-1 (lag crosses block boundaries), so PV weight
                loads never wait on a just-issued exp. pvs[0],pvs[1] share
                a claimed psum bank; pvs[2],pvs[3] own banks."""
                kv = h % 2
                zeros259 = mask_sb[:, 3 * 512:3 * 512 + 259]
                state = {}  # per-block psum tiles, created at first PV

                def drain(b, s, pv, cb):
                    rec = recpool.tile([128, 1], f32,
                                       name=f"rec_{h}_{b}_{s}", tag="rec")
                    nc.vector.reciprocal(rec[:], pv[:, cb + D:cb + D + 1])
                    ot = opool.tile([128, 128], f32,
                                    name=f"ot_{h}_{b}_{s}", tag="ot")
                    nc.vector.tensor_scalar_mul(ot[:], pv[:, cb:cb + D], rec[:])
                    nc.sync.dma_start(
                        out=out_d[b * 512 + s * 128:b * 512 + (s + 1) * 128,
                                  h * D:(h + 1) * D],
                        in_=ot[:])

                def emit_s(b, p):
                    spp = psum2.tile([128, 1024], f32,
                                     name=f"sp_{h}_{b}_{p}", tag="sp2")
                    pt = ptpool.tile([128, 1024], bf16,
                                     name=f"pt_{h}_{b}_{p}", tag="pt")
                    qsl = qt[:, h * T + b * 512:h * T + (b + 1) * 512]
                    for half in range(2):
                        j = 2 * p + half
                        nc.tensor.matmul(
                            spp[:, half * 512:(half + 1) * 512],
                            lhsT=kt[:, kv * T + j * 128:kv * T + (j + 1) * 128],
                            rhs=qsl, start=True, stop=True)
                    nc.scalar.activation(pt[:], spp[:], EXP, scale=SCALE)
                    if p >= 2 * b:
                        roff = (p - 2 * b) * 1024
                        nc.vector.tensor_mul(pt[:], pt[:],
                                             mask_sb[:, roff:roff + 1024])
                    return pt

                def emit_pv(b, p, pt, bi):
                    if b not in state:
                        pvA = psum.tile([128, 512], f32,
                                        name=f"pvA_{h}_{b}", tag="ps")
                        nc.tensor.matmul(pvA[:, 0:259], lhsT=ident,
                                         rhs=zeros259, start=True, stop=False,
                                         skip_group_check=True)
                        pvs23 = [psum.tile([128, 512], f32,
                                           name=f"pv_{h}_{b}_{s}", tag="ps")
                                 for s in (2, 3)]
                        state[b] = (pvA, pvs23)
                    pvA, pvs23 = state[b]
                    for half in range(2):
                        j = 2 * p + half
                        r = j - 4 * b
                        vsl = vt[:, (kv * NT + j) * VROW:
                                 (kv * NT + j + 1) * VROW]
                        for s in range(max(0, r), 4):
                            if s < 2:
                                nc.tensor.matmul(
                                    pvA[:, 130 * s:130 * s + VROW],
                                    lhsT=pt[:, half * 512 + s * 128:
                                            half * 512 + (s + 1) * 128],
                                    rhs=vsl, start=False, stop=False,
                                    skip_group_check=True)
                            else:
                                nc.tensor.matmul(
                                    pvs23[s - 2][:, 0:VROW],
                                    lhsT=pt[:, half * 512 + s * 128:
                                            half * 512 + (s + 1) * 128],
                                    rhs=vsl, start=(j == 0),
                                    stop=(j == 4 * b + s))
                    if p == 2 * b:
                        drain(b, 0, pvA, 0)
                        drain(b, 1, pvA, 130)
                    elif p == 2 * b + 1:
                        drain(b, 2, pvs23[0], 0)
                        drain(b, 3, pvs23[1], 0)
                        del state[b]
                        if hooks and bi in hooks:
                            hooks[bi]()

                steps = [(bi, b, p) for bi, b in enumerate(block_order)
                         for p in range(2 * b + 2)]
                prev = None
                for bi, b, p in steps:
                    pt = emit_s(b, p)
                    if prev is not None:
                        emit_pv(*prev)
                    prev = (b, p, pt, bi)
                    yield
                emit_pv(*prev)
                yield

            def proj(wkey, dest, dbase, vts_kv=None, t4_order=(0, 1, 2, 3)):
                """t4-major projection of one [128, T] strip."""
                w = wts[wkey]
                vts = None
                if vts_kv is not None:
                    vts = vtsp.tile([128, T], bf16, name=f"vts_{wkey}",
                                    tag="vts")
                if True:
                    for t4 in t4_order:
                        ps = psum.tile([128, 512], f32,
                                       name=f"ps_{wkey}_{t4}", tag="ps")
                        for kc in range(KC):
                            nc.tensor.matmul(
                                ps[:], lhsT=w[:, kc * 128:(kc + 1) * 128],
                                rhs=xs(t4, kc),
                                start=(kc == 0), stop=(kc == KC - 1))
                        if vts is None:
                            nc.vector.tensor_copy(
                                out=dest[:, dbase + t4 * 512:
                                         dbase + (t4 + 1) * 512],
                                in_=ps[:])
                        else:
                            nc.vector.tensor_copy(
                                out=vts[:, t4 * 512:(t4 + 1) * 512], in_=ps[:])
                    if vts is not None:
                        for t4 in range(4):
                            emit_tr_group(vts, vts_kv, t4)

            def attn(h, hooks=None, block_order=(0, 1, 2, 3)):
                """Generator: yields after each S/PV pair. pvs[0],pvs[1]
                share one psum bank (claimed by a zeroing matmul so all
                PV accumulations run start=False); pvs[2],pvs[3] own banks."""
                kv = h % 2
                zeros259 = mask_sb[:, 3 * 512:3 * 512 + 259]

                def drain(b, s, pv, cb):
                    rec = recpool.tile([128, 1], f32,
                                       name=f"rec_{h}_{b}_{s}", tag="rec")
                    nc.vector.reciprocal(rec[:], pv[:, cb + D:cb + D + 1])
                    ot = opool.tile([128, 128], f32,
                                    name=f"ot_{h}_{b}_{s}", tag="ot")
                    nc.vector.tensor_scalar_mul(ot[:], pv[:, cb:cb + D], rec[:])
                    nc.sync.dma_start(
                        out=out_d[b * 512 + s * 128:b * 512 + (s + 1) * 128,
                                  h * D:(h + 1) * D],
                        in_=ot[:])

                if True:
                    for bi, b in enumerate(block_order):
                        pvA = psum.tile([128, 512], f32,
                                        name=f"pvA_{h}_{b}", tag="ps")
                        nc.tensor.matmul(pvA[:, 0:259], lhsT=ident,
                                         rhs=zeros259, start=True, stop=False,
                                         skip_group_check=True)
                        pvs23 = [psum.tile([128, 512], f32,
                                           name=f"pv_{h}_{b}_{s}", tag="ps")
                                 for s in (2, 3)]
                        for p in range(2 * b + 2):  # pairs of Tk tiles
                            spp = psum2.tile([128, 1024], f32,
                                             name=f"sp_{h}_{b}_{p}", tag="sp2")
                            pt = ptpool.tile([128, 1024], bf16,
                                             name=f"pt_{h}_{b}_{p}", tag="pt")
                            qsl = qt[:, h * T + b * 512:h * T + (b + 1) * 512]
                            for half in range(2):
                                j = 2 * p + half
                                nc.tensor.matmul(
                                    spp[:, half * 512:(half + 1) * 512],
                                    lhsT=kt[:, kv * T + j * 128:
                                            kv * T + (j + 1) * 128],
                                    rhs=qsl,
                                    start=True, stop=True,
                                )
                            nc.scalar.activation(pt[:], spp[:], EXP,
                                                 scale=SCALE)
                            if p >= 2 * b:  # diagonal pairs
                                roff = (p - 2 * b) * 1024
                                nc.vector.tensor_mul(
                                    pt[:], pt[:],
                                    mask_sb[:, roff:roff + 1024])
                            for half in range(2):
                                j = 2 * p + half
                                r = j - 4 * b
                                vsl = vt[:, (kv * NT + j) * VROW:
                                         (kv * NT + j + 1) * VROW]
                                for s in range(max(0, r), 4):
                                    if s < 2:
                                        nc.tensor.matmul(
                                            pvA[:, 130 * s:130 * s + VROW],
                                            lhsT=pt[:, half * 512 + s * 128:
                                                    half * 512 + (s + 1) * 128],
                                            rhs=vsl,
                                            start=False, stop=False,
                                            skip_group_check=True,
                                        )
                                    else:
                                        nc.tensor.matmul(
                                            pvs23[s - 2][:, 0:VROW],
                                            lhsT=pt[:, half * 512 + s * 128:
                                                    half * 512 + (s + 1) * 128],
                                            rhs=vsl,
                                            start=(j == 0),
                                            stop=(j == 4 * b + s),
                                        )
                            if p == 2 * b:
                                drain(b, 0, pvA, 0)
                                drain(b, 1, pvA, 130)
                            elif p == 2 * b + 1:
                                drain(b, 2, pvs23[0], 0)
                                drain(b, 3, pvs23[1], 0)
                            yield
                        if hooks and bi in hooks:
                            hooks[bi]()

            def proj(wkey, dest, dbase, vts_kv=None, t4_order=(0, 1, 2, 3)):
                """Generator: t4-major projection, yields every 8 kc."""
                w = wts[wkey]
                vts = None
                if vts_kv is not None:
                    vts = vtsp.tile([128, T], bf16, name=f"vts_{wkey}",
                                    tag="vts")
                if True:
                    for t4 in t4_order:
                        ps = psum.tile([128, 512], f32,
                                       name=f"ps_{wkey}_{t4}", tag="ps")
                        for kc in range(KC):
                            nc.tensor.matmul(
                                ps[:], lhsT=w[:, kc * 128:(kc + 1) * 128],
                                rhs=xs(t4, kc),
                                start=(kc == 0), stop=(kc == KC - 1))
                            if kc % 8 == 7 and kc != KC - 1:
                                yield
                        if vts is None:
                            nc.vector.tensor_copy(
                                out=dest[:, dbase + t4 * 512:
                                         dbase + (t4 + 1) * 512],
                                in_=ps[:])
                        else:
                            nc.vector.tensor_copy(
                                out=vts[:, t4 * 512:(t4 + 1) * 512], in_=ps[:])
                        yield
                    if vts is not None:
                        for t4 in range(4):
                            emit_tr_group(vts, vts_kv, t4)
                            yield

            def weave(*streams, until=None):
                """Round-robin weighted interleave. If until is given (index
                into streams), stop as soon as that stream exhausts, leaving
                the other generators unconsumed for later weaves."""
                live = [[g, wt] for g, wt in streams]
                gens = [g for g, _ in streams]
                while live:
                    done = []
                    for ent in live:
                        g, wt = ent
                        for _ in range(wt):
                            try:
                                next(g)
                            except StopIteration:
                                done.append(ent)
                                break
                    for ent in done:
                        live.remove(ent)
                        if until is not None and ent[0] is gens[until]:
                            return

            def chain(*gens):
                for g in gens:
                    yield from g

            # ---- DMA schedule (ring is FIFO in issue order) ----
            # Weight eighths (4 kc each) for the three startup units
            # interleaved with the x chunks that consume them; startup
            # compute is PE-bound from the first chunk.
            E = C // 8
            # h0 x chunk index covering kc group g: chunks 0,1 are kc0-1,2-3
            h0_chunk_for_g = {0: (0, 1), 1: (2,), 2: (3,), 3: (4,),
                              4: (5,), 5: (6,), 6: (7,), 7: (8,)}
            xi_done = set()
            for g in range(8):
                for key, src in (("q0", wq_d), ("k0", wk_d), ("v0", wv_d)):
                    dma_w(src, 0, key, cols=(g * E, (g + 1) * E))
                for ci in h0_chunk_for_g[g]:
                    if ci not in xi_done:
                        dma_x(ci)
                        xi_done.add(ci)
                if g == 3:
                    nc.sync.dma_start(out=mask_sb[:], in_=mask_d[:])
            for ci in range(9, 17):
                dma_x(ci)

            # ---- compute schedule ----
            # attn heads weave only with units they do not depend on;
            # attn3 (blocks 3..0) weaves with q3's t4 1,0 after t4 3,2
            # completed during the attn1 weave.
            startup()
            dma_w(wq_d, 2, "q2")
            units_a = chain(proj("q2", qt, 2 * T), proj("k1", kt, T))
            weave((attn(0, hooks={1: lambda: dma_w(wk_d, 1, "k1"),
                                  2: lambda: dma_w(wv_d, 1, "v1")}), 1),
                  (units_a, 1), until=0)
            dma_w(wq_d, 1, "q1")
            weave((attn(2, hooks={2: lambda: dma_w(wq_d, 3, "q3")}), 1),
                  (chain(units_a,
                         proj("v1", None, 0, vts_kv=1),
                         proj("q1", qt, T)), 2))
            weave((attn(1), 2),
                  (proj("q3", qt, 3 * T, t4_order=(3, 2)), 1))
            weave((attn(3, block_order=(3, 2, 1, 0)), 2),
                  (proj("q3", qt, 3 * T, t4_order=(1, 0)), 1))

    nc.compile()
    _prog_cache["nc"] = nc
    return nc


def _host_prep(x, Wq, bq, Wk, bk, Wv, bv):
    """Shard + repack inputs for the 8 cores. Returns in_maps list."""
    assert x.shape == (1, T, C)
    assert np.abs(bq).max() == 0 and np.abs(bk).max() == 0, \
        "nonzero q/k biases not supported"

    x0 = np.ascontiguousarray(x[0]).astype(BF16)
    # xh packed: [128, half*32K + kc*1024 + t'] = x[half*1024+t', kc*128+p]
    xh = np.ascontiguousarray(
        x0.reshape(2, 1024, KC, 128).transpose(3, 0, 2, 1).reshape(128, XCOLS))

    # causal masks for the 4 diagonal-tile offsets: mask_r[tk,tq] = tq >= tk+128r
    tq = np.arange(512)[None, :]
    tk = np.arange(128)[:, None]
    masks = np.concatenate(
        [(tq >= (tk + 128 * r)).astype(BF16) for r in range(4)]
        + [np.eye(128, dtype=BF16)], axis=1)
    masks = np.ascontiguousarray(masks)

    def pack_w(Wrows):
        # Wrows: [128 (out c), C (in)] for one head ->
        # packed[p, 128*kc + c] = Wrows[c, 128*kc + p]
        return np.ascontiguousarray(
            Wrows.astype(BF16).reshape(128, KC, 128).transpose(2, 1, 0)
            .reshape(128, C))

    in_maps = []
    for c in range(N_CORES):
        qheads = [2 * c, 2 * c + 1, 2 * c + 16, 2 * c + 17]
        kvheads = [2 * c, 2 * c + 1]
        wq = np.stack([pack_w(Wq[128 * H:128 * (H + 1)]) for H in qheads])
        wk = np.stack([pack_w(Wk[128 * K:128 * (K + 1)]) for K in kvheads])
        wv = np.stack([pack_w(Wv[128 * K:128 * (K + 1)]) for K in kvheads])
        in_maps.append({
            "xh": xh, "wq": wq, "wk": wk, "wv": wv, "masks": masks,
        })
    return in_maps


def _assemble(results, bv):
    out = np.empty((T, C), dtype=np.float32)
    for c in range(N_CORES):
        r = results[c]["out"]
        qheads = [2 * c, 2 * c + 1, 2 * c + 16, 2 * c + 17]
        for i, H in enumerate(qheads):
            blk = r[:, 128 * i:128 * (i + 1)]
            if bv is not None:
                blk = blk + bv[128 * (H % N_KV_HEADS):
                               128 * (H % N_KV_HEADS) + 128]
            out[:, 128 * H:128 * (H + 1)] = blk
    return out.reshape(1, T, C)


def _install_trace_hooks():
    """The agent image's antenv lacks axon_hooks; recreate it so
    run_bass_kernel_spmd's trace=True path can capture NTFF profiles."""
    import sys
    import types
    import antenv
    if "antenv.axon_hooks" not in sys.modules:
        mod = types.ModuleType("antenv.axon_hooks")
        mod._hook = None

        def set_axon_ntff_profile_hook(h):
            mod._hook = h

        def get_axon_ntff_profile_hook():
            return mod._hook

        mod.set_axon_ntff_profile_hook = set_axon_ntff_profile_hook
        mod.get_axon_ntff_profile_hook = get_axon_ntff_profile_hook
        sys.modules["antenv.axon_hooks"] = mod
        antenv.axon_hooks = mod
    from antenv.axon_hooks import (get_axon_ntff_profile_hook,
                                   set_axon_ntff_profile_hook)
    if get_axon_ntff_profile_hook() is None:
        if "/root/.axon_site" not in sys.path:
            sys.path.insert(0, "/root/.axon_site")
        from trn_agent_boot.trn_boot import _ntff_profile_via_ctypes
        set_axon_ntff_profile_hook(
            _ntff_profile_via_ctypes("/opt/axon/libaxon_pjrt.so"))
    import concourse.bass_utils as bu
    bu.upload_artifacts = lambda tmpdir: tmpdir


def _run(inputs, trace=False, trace_kwargs=None):
    if trace:
        _install_trace_hooks()
    from concourse.bass_utils import run_bass_kernel_spmd
    nc = _build_program()
    in_maps = _host_prep(**inputs)
    res = run_bass_kernel_spmd(
        nc, in_maps, list(range(N_CORES)), trace=trace,
        **(trace_kwargs or {}))
    bv = inputs["bv"].astype(np.float32)
    bv = bv if np.abs(bv).max() > 0 else None
    out = _assemble(res.results, bv)
    return out, res


def kernel(x, Wq, bq, Wk, bk, Wv, bv):
    out, _ = _run(dict(x=np.asarray(x), Wq=np.asarray(Wq), bq=np.asarray(bq),
                       Wk=np.asarray(Wk), bk=np.asarray(bk),
                       Wv=np.asarray(Wv), bv=np.asarray(bv)))
    return out
